# revision 3
# baseline (speedup 1.0000x reference)
"""Multi-head GAT layer on 8 Trainium2 NeuronCores.

Strategy (one SPMD program, 8 cores; per-core differences are data only):
  - Destination nodes are split into 8 contiguous, 128-aligned ranges
    balanced by edge count.  Each core computes out[] rows for its range.
  - Each core's inputs are supplied in a per-core *node permutation* that
    places its own (local) nodes first; all edge indices are pre-translated
    on the host (dma_gather indices are int16, hence a lo/hi table split).
  - Every core computes the full projection table with the tensor engine
    (replicated; avoids any collective):
       fat row [node, 384 bf16] = [xh bf16 (512B) | s_src f32 (16B) | pad]
    The per-node s_dst scores stay resident in SBUF (never round-trip DRAM).
  - Edges (with self loops) are grouped by destination tile (128 dsts) and
    padded to 128-edge chunks (one edge per SBUF partition).  Per chunk:
       G    = dma_gather(fat_table, src)          # 768B rows, the ONLY
                                                  # gpsimd-descriptor work
       sd_e = PT_chunk^T @ sdst_tile              # per-edge s_dst via a
                                                  # host-streamed one-hot
       w    = exp(leaky_relu(sd_e + G.s_src)) -> written into G[256:260]
       G[:, 0:256] *= w (bcast over 64)
       psum[128d, 260] += P_chunk^T @ G[:, 0:260] # host-streamed one-hot P
    Columns 0:256 accumulate the numerator, 256:260 the denominator
    (w columns summed by the same one-hot matmul).  out = num/den.
  - P (bf16) and PT (f32) matrices are precomputed on the host from the
    edge list and streamed as plain contiguous DMA: no on-device index
    math, no per-edge descriptor generation beyond the single fat gather.
  - exp() needs no max subtraction: scores are O(1) (inputs are standard
    normal), so overflow is impossible; softmax is shift-invariant.
"""

import sys

sys.path.insert(0, "/opt/trn_rl_repo")

import numpy as np
import ml_dtypes

import concourse.bass as bass
import concourse.mybir as mybir
from concourse import bacc, tile
from concourse.bass_utils import run_bass_kernel_spmd

# Problem constants (hardcoded per contest rules).
N_NODES = 50000
CIN = 128
COUT = 64
H = 4
HC = H * COUT  # 256
FATU = 384  # fat row in bf16/u16 units: xh(256) | s_src f32 as 8 u16 | pad
NEG_SLOPE = 0.2

NCORES = 8
TILE = 128
LO = 32768
NPAD = ((N_NODES + TILE - 1) // TILE) * TILE  # 50048
NTILES_A = NPAD // TILE  # 391
GROUP = 2  # dst tiles per phase-C group
GROUP_A = 16  # projection tiles per staging group

F32 = mybir.dt.float32
F32R = mybir.dt.float32r
BF16 = mybir.dt.bfloat16
I16 = mybir.dt.int16


def _cdiv(a, b):
    return (a + b - 1) // b


class Structure:
    def __init__(self, T, CL, CH, groups, TC):
        self.T = T
        self.CL = CL
        self.CH = CH
        self.groups = groups  # (tiles, fc_lo0, nlo, fc_hi0, nhi)
        self.TC = TC


def _preprocess(edge_index):
    """Host-side integer-only preprocessing."""
    src = edge_index[0].astype(np.int64)
    dst = edge_index[1].astype(np.int64)
    loops = np.arange(N_NODES, dtype=np.int64)
    row = np.concatenate([src, loops])
    col = np.concatenate([dst, loops])
    order = np.argsort(col, kind="stable")
    row = row[order]
    col = col[order]
    counts = np.bincount(col, minlength=N_NODES)
    ccum = np.concatenate([[0], np.cumsum(counts)])
    etot = row.size

    bounds = [0]
    for c in range(1, NCORES):
        n = int(np.searchsorted(ccum, etot * c // NCORES))
        n = (n // TILE) * TILE
        bounds.append(min(max(n, bounds[-1] + TILE), N_NODES - TILE))
    bounds.append(N_NODES)
    nloc = [bounds[c + 1] - bounds[c] for c in range(NCORES)]
    T = max(_cdiv(nl, TILE) for nl in nloc)

    # Per core: permuted-src + local-dst-offset edge lists per tile/pass.
    per_core = []
    for c in range(NCORES):
        n0, n1 = bounds[c], bounds[c + 1]
        e0, e1 = int(ccum[n0]), int(ccum[n1])
        r = row[e0:e1]
        d = col[e0:e1] - n0
        nl = n1 - n0
        # permuted source id: local nodes first, then [0,n0), then [n1,N)
        pr = np.where(
            (r >= n0) & (r < n1),
            r - n0,
            np.where(r < n0, nl + r, nl + n0 + (r - n1)),
        )
        tt = d // TILE
        tiles = []
        for t in range(T):
            m = tt == t
            rs = pr[m]
            ds = d[m] - t * TILE
            lom = rs < LO
            tiles.append((rs[lom], ds[lom], rs[~lom] - LO, ds[~lom]))
        per_core.append(tiles)

    CL = [
        max(_cdiv(len(per_core[c][t][0]), TILE) for c in range(NCORES))
        for t in range(T)
    ]
    CH = [
        max(_cdiv(len(per_core[c][t][2]), TILE) for c in range(NCORES))
        for t in range(T)
    ]
    for t in range(T):
        if CL[t] + CH[t] == 0:
            CL[t] = 1

    groups = []
    fc = 0
    for g0 in range(0, T, GROUP):
        tiles = list(range(g0, min(g0 + GROUP, T)))
        nlo = sum(CL[t] for t in tiles)
        nhi = sum(CH[t] for t in tiles)
        groups.append((tiles, fc, nlo, fc + nlo, nhi))
        fc += nlo + nhi
    TC = fc

    st = Structure(T, CL, CH, groups, TC)

    gidx = np.zeros((NCORES, 128, TC * 8), np.int16)  # by permuted src
    # One-hot scatter (P) / broadcast (PT) matrices, streamed to the device.
    #   Pm[c][e, fc*128 + d] = 1  where chunk fc's edge e targets dst-off d
    #   PTm[c][d, fc*128 + e] = 1 (transpose layout, partition = dst-off)
    Pm = np.zeros((NCORES, 128, TC * 128), ml_dtypes.bfloat16)
    PTm = np.zeros((NCORES, 128, TC * 128), np.float32)

    wrow = np.arange(128) % 16
    wcol = np.arange(128) // 16

    for c in range(NCORES):
        p_e, p_col = [], []  # Pm fancy-index accumulators
        pt_d, pt_col = [], []
        for tiles, fc_lo0, nlo, fc_hi0, nhi in st.groups:
            fl, fh = fc_lo0, fc_hi0
            for t in tiles:
                lo_s, lo_d, hi_s, hi_d = per_core[c][t]
                for passno in range(2):
                    s_arr, d_arr = (lo_s, lo_d) if passno == 0 else (hi_s, hi_d)
                    nch = CL[t] if passno == 0 else CH[t]
                    base = fl if passno == 0 else fh
                    for k in range(nch):
                        sl = slice(k * TILE, (k + 1) * TILE)
                        sv = s_arr[sl]
                        dv = d_arr[sl]
                        m = len(sv)
                        fcx = base + k
                        if m:
                            for rep in range(8):
                                rr = rep * 16 + wrow[:m]
                                cc = fcx * 8 + wcol[:m]
                                gidx[c, rr, cc] = sv.astype(np.int16)
                            ep = np.arange(m)
                            p_e.append(ep)
                            p_col.append(fcx * 128 + dv)
                            pt_d.append(dv)
                            pt_col.append(fcx * 128 + ep)
                    if passno == 0:
                        fl += nch
                    else:
                        fh += nch
        if p_e:
            Pm[c][np.concatenate(p_e), np.concatenate(p_col)] = 1
            PTm[c][np.concatenate(pt_d), np.concatenate(pt_col)] = 1

    return st, bounds, nloc, gidx, Pm, PTm


def _build_program(st):
    import os

    stage = os.environ.get("GAT_STAGE", "full")
    nc = bacc.Bacc(None, target_bir_lowering=False)
    TC = st.TC
    T = st.T

    xT_in = nc.dram_tensor("xT", [128, NPAD], F32R, kind="ExternalInput")
    wt_in = nc.dram_tensor("Wt", [128, HC], F32, kind="ExternalInput")
    arep_in = nc.dram_tensor("arep", [128, 2 * HC], F32, kind="ExternalInput")
    gidx_in = nc.dram_tensor("gidx", [128, TC * 8], I16, kind="ExternalInput")
    pm_in = nc.dram_tensor("Pm", [128, TC * 128], BF16, kind="ExternalInput")
    ptm_in = nc.dram_tensor("PTm", [128, TC * 128], F32R, kind="ExternalInput")
    y_out = nc.dram_tensor("y", [T * 128, HC], F32, kind="ExternalOutput")

    with tile.TileContext(nc) as tc:
        with (
            tc.tile_pool(name="dram", bufs=1, space="DRAM") as dram,
            tc.tile_pool(name="persist", bufs=1) as pp,
        ):
            xh_t = dram.tile([NPAD, FATU], BF16)

            wt_ext = pp.tile([128, HC + 8], F32R)
            gidx_s = pp.tile([128, TC * 8], I16)
            nc.sync.dma_start(gidx_s[:], gidx_in[:])
            sdst_sb = pp.tile([128, T, 4], F32R)

            # ---- Wt_ext = [Wt | v_src(4) | v_dst(4)] -----------------------
            with tc.tile_pool(name="winit", bufs=1) as wini:
                wtile = wini.tile([128, HC], F32)
                nc.sync.dma_start(wtile[:], wt_in[:])
                arep_s = wini.tile([128, 2 * HC], F32)
                nc.sync.dma_start(arep_s[:], arep_in[:])
                tmp = wini.tile([128, HC], F32)
                wt_f = wini.tile([128, HC + 8], F32)
                nc.vector.tensor_copy(wt_f[:, 0:HC], wtile[:])
                # cols 256:260 = s_src (a_j), cols 260:264 = s_dst (a_i)
                nc.vector.tensor_mul(tmp[:], wtile[:], arep_s[:, HC : 2 * HC])
                for h in range(H):
                    nc.vector.tensor_reduce(
                        wt_f[:, HC + h : HC + h + 1],
                        tmp[:, h * COUT : (h + 1) * COUT],
                        mybir.AxisListType.X,
                        mybir.AluOpType.add,
                    )
                nc.vector.tensor_mul(tmp[:], wtile[:], arep_s[:, 0:HC])
                for h in range(H):
                    nc.vector.tensor_reduce(
                        wt_f[:, HC + 4 + h : HC + 5 + h],
                        tmp[:, h * COUT : (h + 1) * COUT],
                        mybir.AxisListType.X,
                        mybir.AluOpType.add,
                    )
                nc.vector.tensor_copy(wt_ext[:], wt_f[:])

            # ---- Phase A: projection table ---------------------------------
            fat_v = xh_t.rearrange("(t p) f -> p t f", p=128)
            with (
                tc.tile_pool(name="pha", bufs=2) as pa,
                tc.tile_pool(name="psA", bufs=4, space="PSUM") as psA,
            ):
                for g in range(0, NTILES_A, GROUP_A):
                    gt = min(GROUP_A, NTILES_A - g)
                    stA = pa.tile([128, gt, FATU], BF16, tag="stA")
                    xt = None
                    for i in range(gt):
                        if i % 4 == 0:
                            xt = pa.tile([128, 4 * 128], F32R, tag="xt")
                            g0 = (g + i) * 128
                            xw = min(4 * 128, NPAD - g0)
                            nc.sync.dma_start(
                                xt[:, 0:xw], xT_in[:, g0 : g0 + xw]
                            )
                        ps = psA.tile([128, HC + 8], F32, tag="psA")
                        nc.tensor.matmul(
                            ps[:],
                            xt[:, (i % 4) * 128 : (i % 4 + 1) * 128],
                            wt_ext[:],
                            start=True,
                            stop=True,
                        )
                        # psum: [xh(0:256) | s_src(256:260) | s_dst(260:264)]
                        if i % 2 == 0:
                            nc.vector.tensor_copy(
                                stA[:, i, 0:HC], ps[:, 0:HC]
                            )
                        else:
                            nc.scalar.copy(stA[:, i, 0:HC], ps[:, 0:HC])
                        # s_src f32 bits into u16 cols 256:264 (raw f32 copy
                        # through a bitcast view: no conversion)
                        nc.vector.tensor_copy(
                            stA[:, i, HC : HC + 8].bitcast(F32),
                            ps[:, HC : HC + 4],
                        )
                        # finite filler for gathered-but-unused tail cols
                        nc.scalar.copy(
                            stA[:, i, HC + 8 : FATU].bitcast(F32),
                            ps[:, HC - 60 : HC],
                        )
                        # per-node s_dst stays resident in SBUF
                        if g + i < T:
                            nc.vector.tensor_copy(
                                sdst_sb[:, g + i, :], ps[:, HC + 4 : HC + 8]
                            )
                    nc.sync.dma_start(fat_v[:, g : g + gt, :], stA[:])

            # ---- Phase C: gather + scores + one-hot scatter matmul ---------
            y_v = y_out.rearrange("(t p) f -> p t f", p=128)
            if stage == "A":
                with tc.tile_pool(name="ost0", bufs=2) as ost0:
                    for tiles, fc_lo0, nlo, fc_hi0, nhi in st.groups:
                        og = ost0.tile([128, len(tiles), HC], F32, tag="og")
                        nc.vector.memset(og[:], 0.0)
                        nc.sync.dma_start(
                            y_v[:, tiles[0] : tiles[0] + len(tiles), :], og[:]
                        )
                nc.compile()
                return nc
            with (
                tc.tile_pool(name="phc", bufs=2) as pc,
                tc.tile_pool(name="mk", bufs=4) as mk,
                tc.tile_pool(name="pssg", bufs=2, space="PSUM") as psg,
                tc.tile_pool(name="pso", bufs=2, space="PSUM") as pso,
                tc.tile_pool(name="ost", bufs=2) as ost,
            ):
                for tiles, fc_lo0, nlo, fc_hi0, nhi in st.groups:
                    nall = nlo + nhi
                    # flat chunk -> tile map (lo block then hi block)
                    tflat = [t for t in tiles for _ in range(st.CL[t])] + [
                        t for t in tiles for _ in range(st.CH[t])
                    ]
                    # one-hot streams for this group's flat chunk range
                    Ps = pc.tile([128, nall * 128], BF16, tag="Ps")
                    nc.sync.dma_start(
                        Ps[:],
                        pm_in[:, fc_lo0 * 128 : (fc_lo0 + nall) * 128],
                    )
                    PTs = pc.tile([128, nall * 128], F32R, tag="PTs")
                    nc.sync.dma_start(
                        PTs[:],
                        ptm_in[:, fc_lo0 * 128 : (fc_lo0 + nall) * 128],
                    )
                    parts = []
                    if nlo:
                        glo = pc.tile([128, nlo, FATU], BF16, tag="glo")
                        nc.gpsimd.dma_gather(
                            glo[:],
                            xh_t[:, :],
                            gidx_s[:, fc_lo0 * 8 : (fc_lo0 + nlo) * 8],
                            nlo * 128,
                            nlo * 128,
                            FATU,
                            single_packet=False,
                        )
                        parts.append((glo, 0, nlo))
                    if nhi:
                        ghi = pc.tile([128, nhi, FATU], BF16, tag="ghi")
                        nc.gpsimd.dma_gather(
                            ghi[:],
                            xh_t[LO:, :],
                            gidx_s[:, fc_hi0 * 8 : (fc_hi0 + nhi) * 8],
                            nhi * 128,
                            nhi * 128,
                            FATU,
                            single_packet=False,
                        )
                        parts.append((ghi, nlo, nhi))

                    # per-edge s_dst via one-hot transpose matmuls
                    pssg = psg.tile([128, nall, 4], F32, tag="pssg")
                    for j in range(nall):
                        nc.tensor.matmul(
                            pssg[:, j, :],
                            PTs[:, j * 128 : (j + 1) * 128],
                            sdst_sb[:, tflat[j], :],
                            start=True,
                            stop=True,
                            skip_group_check=True,
                        )

                    eg = pc.tile([128, nall, 4], F32, tag="eg")
                    for gt_, ofs, nch in parts:
                        # e = s_dst[dst] + s_src[src]
                        nc.vector.tensor_add(
                            eg[:, ofs : ofs + nch, :],
                            pssg[:, ofs : ofs + nch, :],
                            gt_[:, :, HC : HC + 8].bitcast(F32),
                        )
                    # leaky_relu: (e*0.2) max e
                    nc.vector.scalar_tensor_tensor(
                        eg[:],
                        eg[:],
                        NEG_SLOPE,
                        eg[:],
                        mybir.AluOpType.mult,
                        mybir.AluOpType.max,
                    )
                    for gt_, ofs, nch in parts:
                        # w = exp(e), written into G cols 256:260 (over the
                        # no-longer-needed s_src bits)
                        nc.scalar.activation(
                            gt_[:, :, HC : HC + 4],
                            eg[:, ofs : ofs + nch, :],
                            mybir.ActivationFunctionType.Exp,
                        )
                        # G[:, 0:256] *= w (bcast over 64)
                        nc.vector.tensor_mul(
                            gt_[:, :, 0:HC].rearrange(
                                "p c (h o) -> p c h o", o=COUT
                            ),
                            gt_[:, :, 0:HC].rearrange(
                                "p c (h o) -> p c h o", o=COUT
                            ),
                            gt_[:, :, HC : HC + 4]
                            .unsqueeze(-1)
                            .broadcast_to([128, nch, 4, COUT]),
                        )

                    og = ost.tile([128, len(tiles), HC], F32, tag="og")
                    lo_j = 0
                    hi_j = 0
                    for ti, t in enumerate(tiles):
                        ps = pso.tile([128, HC + 4], F32, tag="ps")
                        K = st.CL[t] + st.CH[t]
                        k = 0
                        for passno in range(2):
                            nch = st.CL[t] if passno == 0 else st.CH[t]
                            for _ in range(nch):
                                if passno == 0:
                                    flat = lo_j
                                    gsl = glo[:, lo_j, 0 : HC + 4]
                                    lo_j += 1
                                else:
                                    flat = nlo + hi_j
                                    gsl = ghi[:, hi_j, 0 : HC + 4]
                                    hi_j += 1
                                nc.tensor.matmul(
                                    ps[:],
                                    Ps[:, flat * 128 : (flat + 1) * 128],
                                    gsl,
                                    start=(k == 0),
                                    stop=(k == K - 1),
                                    skip_group_check=True,
                                )
                                k += 1
                        den = mk.tile([128, 4], F32, tag="den")
                        nc.vector.tensor_scalar(
                            den[:],
                            ps[:, HC : HC + 4],
                            1e-30,
                            None,
                            mybir.AluOpType.add,
                        )
                        rec = mk.tile([128, 4], F32, tag="rec")
                        nc.vector.reciprocal(rec[:], den[:])
                        nc.vector.tensor_mul(
                            og[:, ti, :].rearrange("p (h o) -> p h o", o=COUT),
                            ps[:, 0:HC].rearrange("p (h o) -> p h o", o=COUT),
                            rec.unsqueeze(-1).broadcast_to([128, 4, COUT]),
                        )
                    nc.sync.dma_start(
                        y_v[:, tiles[0] : tiles[0] + len(tiles), :], og[:]
                    )

    nc.compile()
    return nc


def _make_in_maps(st, bounds, x, W, a, gidx, Pm, PTm):
    xt_g = np.zeros((128, NPAD), np.float32)
    xt_g[:, :N_NODES] = np.ascontiguousarray(x.T)
    Wt = np.ascontiguousarray(W.transpose(2, 0, 1).reshape(CIN, HC)).astype(
        np.float32
    )
    arep = np.tile(
        np.concatenate([a[:, :COUT].reshape(-1), a[:, COUT:].reshape(-1)])[
            None, :
        ],
        (128, 1),
    ).astype(np.float32)

    in_maps = []
    for c in range(NCORES):
        n0, n1 = bounds[c], bounds[c + 1]
        nl = n1 - n0
        xTc = np.empty((128, NPAD), np.float32)
        xTc[:, :nl] = xt_g[:, n0:n1]
        xTc[:, nl : nl + n0] = xt_g[:, 0:n0]
        xTc[:, nl + n0 : nl + n0 + (NPAD - n1)] = xt_g[:, n1:NPAD]
        in_maps.append(
            {
                "xT": xTc,
                "Wt": Wt,
                "arep": arep,
                "gidx": np.ascontiguousarray(gidx[c]),
                "Pm": np.ascontiguousarray(Pm[c]),
                "PTm": np.ascontiguousarray(PTm[c]),
            }
        )
    return in_maps


_CACHE = {}


def _get_compiled(edge_key, edge_index):
    if edge_key not in _CACHE:
        st, bounds, nloc, gidx, Pm, PTm = _preprocess(edge_index)
        nc = _build_program(st)
        _CACHE[edge_key] = (st, bounds, nloc, gidx, Pm, PTm, nc)
    return _CACHE[edge_key]


def kernel(x, edge_index, W, a, num_nodes, _trace=False):
    x = np.asarray(x)
    edge_index = np.asarray(edge_index)
    W = np.asarray(W)
    a = np.asarray(a)

    edge_key = hash(edge_index.tobytes())
    st, bounds, nloc, gidx, Pm, PTm, nc = _get_compiled(edge_key, edge_index)
    in_maps = _make_in_maps(st, bounds, x, W, a, gidx, Pm, PTm)

    kw = {}
    if _trace:
        kw = dict(trace=True)
    res = run_bass_kernel_spmd(nc, in_maps, core_ids=list(range(NCORES)), **kw)

    out = np.empty((N_NODES, HC), np.float32)
    for c in range(NCORES):
        y = res.results[c]["y"]
        out[bounds[c] : bounds[c + 1]] = y[: nloc[c]]
    if _trace:
        return out, res
    return out


# revision 4
# speedup vs baseline: 1.0084x; 1.0084x over previous
"""Multi-head GAT layer on 8 Trainium2 NeuronCores.

Strategy (one SPMD program, 8 cores; per-core differences are data only):
  - Destination nodes are split into 8 contiguous, 128-aligned ranges
    balanced by edge count.  Each core computes out[] rows for its range.
  - Each core's inputs are supplied in a per-core *node permutation* that
    places its own (local) nodes first; all edge indices are pre-translated
    on the host (dma_gather indices are int16, hence a lo/hi table split).
  - Every core computes the full projection table with the tensor engine
    (replicated; avoids any collective):
       fat row [node, 384 bf16] = [xh bf16 (512B) | s_src f32 (16B) | pad]
    The per-node s_dst scores stay resident in SBUF (never round-trip DRAM).
  - Edges (with self loops) are grouped by destination tile (128 dsts) and
    padded to 128-edge chunks (one edge per SBUF partition).  Per chunk:
       G    = dma_gather(fat_table, src)          # 768B rows, the ONLY
                                                  # gpsimd-descriptor work
       sd_e = PT_chunk^T @ sdst_tile              # per-edge s_dst via a
                                                  # host-streamed one-hot
       w    = exp(leaky_relu(sd_e + G.s_src)) -> written into G[256:260]
       G[:, 0:256] *= w (bcast over 64)
       psum[128d, 260] += P_chunk^T @ G[:, 0:260] # host-streamed one-hot P
    Columns 0:256 accumulate the numerator, 256:260 the denominator
    (w columns summed by the same one-hot matmul).  out = num/den.
  - P (bf16) and PT (f32) matrices are precomputed on the host from the
    edge list and streamed as plain contiguous DMA: no on-device index
    math, no per-edge descriptor generation beyond the single fat gather.
  - exp() needs no max subtraction: scores are O(1) (inputs are standard
    normal), so overflow is impossible; softmax is shift-invariant.
"""

import sys

sys.path.insert(0, "/opt/trn_rl_repo")

import numpy as np
import ml_dtypes

import concourse.bass as bass
import concourse.mybir as mybir
from concourse import bacc, tile
from concourse.bass_utils import run_bass_kernel_spmd

# Problem constants (hardcoded per contest rules).
N_NODES = 50000
CIN = 128
COUT = 64
H = 4
HC = H * COUT  # 256
FATU = 384  # fat row in bf16/u16 units: xh(256) | s_src f32 as 8 u16 | pad
NEG_SLOPE = 0.2

NCORES = 8
TILE = 128
LO = 32768
NPAD = ((N_NODES + TILE - 1) // TILE) * TILE  # 50048
NTILES_A = NPAD // TILE  # 391
GROUP = 2  # dst tiles per phase-C group
GROUP_A = 16  # projection tiles per staging group

F32 = mybir.dt.float32
F32R = mybir.dt.float32r
BF16 = mybir.dt.bfloat16
I16 = mybir.dt.int16


def _cdiv(a, b):
    return (a + b - 1) // b


class Structure:
    def __init__(self, T, CL, CH, groups, TC):
        self.T = T
        self.CL = CL
        self.CH = CH
        self.groups = groups  # (tiles, fc_lo0, nlo, fc_hi0, nhi)
        self.TC = TC


def _preprocess(edge_index):
    """Host-side integer-only preprocessing."""
    src = edge_index[0].astype(np.int64)
    dst = edge_index[1].astype(np.int64)
    loops = np.arange(N_NODES, dtype=np.int64)
    row = np.concatenate([src, loops])
    col = np.concatenate([dst, loops])
    order = np.argsort(col, kind="stable")
    row = row[order]
    col = col[order]
    counts = np.bincount(col, minlength=N_NODES)
    ccum = np.concatenate([[0], np.cumsum(counts)])
    etot = row.size

    bounds = [0]
    for c in range(1, NCORES):
        n = int(np.searchsorted(ccum, etot * c // NCORES))
        n = (n // TILE) * TILE
        bounds.append(min(max(n, bounds[-1] + TILE), N_NODES - TILE))
    bounds.append(N_NODES)
    nloc = [bounds[c + 1] - bounds[c] for c in range(NCORES)]
    T = max(_cdiv(nl, TILE) for nl in nloc)

    # Per core: permuted-src + local-dst-offset edge lists per tile/pass.
    per_core = []
    for c in range(NCORES):
        n0, n1 = bounds[c], bounds[c + 1]
        e0, e1 = int(ccum[n0]), int(ccum[n1])
        r = row[e0:e1]
        d = col[e0:e1] - n0
        nl = n1 - n0
        # permuted source id: local nodes first, then [0,n0), then [n1,N)
        pr = np.where(
            (r >= n0) & (r < n1),
            r - n0,
            np.where(r < n0, nl + r, nl + n0 + (r - n1)),
        )
        tt = d // TILE
        tiles = []
        for t in range(T):
            m = tt == t
            rs = pr[m]
            ds = d[m] - t * TILE
            lom = rs < LO
            tiles.append((rs[lom], ds[lom], rs[~lom] - LO, ds[~lom]))
        per_core.append(tiles)

    CL = [
        max(_cdiv(len(per_core[c][t][0]), TILE) for c in range(NCORES))
        for t in range(T)
    ]
    CH = [
        max(_cdiv(len(per_core[c][t][2]), TILE) for c in range(NCORES))
        for t in range(T)
    ]
    for t in range(T):
        if CL[t] + CH[t] == 0:
            CL[t] = 1

    groups = []
    fc = 0
    for g0 in range(0, T, GROUP):
        tiles = list(range(g0, min(g0 + GROUP, T)))
        nlo = sum(CL[t] for t in tiles)
        nhi = sum(CH[t] for t in tiles)
        groups.append((tiles, fc, nlo, fc + nlo, nhi))
        fc += nlo + nhi
    TC = fc

    st = Structure(T, CL, CH, groups, TC)

    gidx = np.zeros((NCORES, 128, TC * 8), np.int16)  # by permuted src
    # One-hot scatter (P) / broadcast (PT) matrices, streamed to the device.
    #   Pm[c][e, fc*128 + d] = 1  where chunk fc's edge e targets dst-off d
    #   PTm[c][d, fc*128 + e] = 1 (transpose layout, partition = dst-off)
    Pm = np.zeros((NCORES, 128, TC * 128), ml_dtypes.bfloat16)
    PTm = np.zeros((NCORES, 128, TC * 128), np.float32)

    wrow = np.arange(128) % 16
    wcol = np.arange(128) // 16

    for c in range(NCORES):
        p_e, p_col = [], []  # Pm fancy-index accumulators
        pt_d, pt_col = [], []
        for tiles, fc_lo0, nlo, fc_hi0, nhi in st.groups:
            fl, fh = fc_lo0, fc_hi0
            for t in tiles:
                lo_s, lo_d, hi_s, hi_d = per_core[c][t]
                for passno in range(2):
                    s_arr, d_arr = (lo_s, lo_d) if passno == 0 else (hi_s, hi_d)
                    nch = CL[t] if passno == 0 else CH[t]
                    base = fl if passno == 0 else fh
                    for k in range(nch):
                        sl = slice(k * TILE, (k + 1) * TILE)
                        sv = s_arr[sl]
                        dv = d_arr[sl]
                        m = len(sv)
                        fcx = base + k
                        if m:
                            for rep in range(8):
                                rr = rep * 16 + wrow[:m]
                                cc = fcx * 8 + wcol[:m]
                                gidx[c, rr, cc] = sv.astype(np.int16)
                            ep = np.arange(m)
                            p_e.append(ep)
                            p_col.append(fcx * 128 + dv)
                            pt_d.append(dv)
                            pt_col.append(fcx * 128 + ep)
                    if passno == 0:
                        fl += nch
                    else:
                        fh += nch
        if p_e:
            Pm[c][np.concatenate(p_e), np.concatenate(p_col)] = 1
            PTm[c][np.concatenate(pt_d), np.concatenate(pt_col)] = 1

    return st, bounds, nloc, gidx, Pm, PTm


def _build_program(st):
    import os

    stage = os.environ.get("GAT_STAGE", "full")
    nc = bacc.Bacc(None, target_bir_lowering=False)
    TC = st.TC
    T = st.T

    xT_in = nc.dram_tensor("xT", [128, NPAD], F32R, kind="ExternalInput")
    wt_in = nc.dram_tensor("Wt", [128, HC], F32, kind="ExternalInput")
    arep_in = nc.dram_tensor("arep", [128, 2 * HC], F32, kind="ExternalInput")
    gidx_in = nc.dram_tensor("gidx", [128, TC * 8], I16, kind="ExternalInput")
    pm_in = nc.dram_tensor("Pm", [128, TC * 128], BF16, kind="ExternalInput")
    ptm_in = nc.dram_tensor("PTm", [128, TC * 128], F32R, kind="ExternalInput")
    y_out = nc.dram_tensor("y", [T * 128, HC], F32, kind="ExternalOutput")

    with tile.TileContext(nc) as tc:
        with (
            tc.tile_pool(name="dram", bufs=1, space="DRAM") as dram,
            tc.tile_pool(name="persist", bufs=1) as pp,
        ):
            xh_t = dram.tile([NPAD, FATU], BF16)

            wt_ext = pp.tile([128, HC + 8], F32R)
            gidx_s = pp.tile([128, TC * 8], I16)
            nc.sync.dma_start(gidx_s[:], gidx_in[:])
            sdst_sb = pp.tile([128, T, 4], F32R)

            # ---- Wt_ext = [Wt | v_src(4) | v_dst(4)] -----------------------
            with tc.tile_pool(name="winit", bufs=1) as wini:
                wtile = wini.tile([128, HC], F32)
                nc.sync.dma_start(wtile[:], wt_in[:])
                arep_s = wini.tile([128, 2 * HC], F32)
                nc.sync.dma_start(arep_s[:], arep_in[:])
                tmp = wini.tile([128, HC], F32)
                wt_f = wini.tile([128, HC + 8], F32)
                nc.vector.tensor_copy(wt_f[:, 0:HC], wtile[:])
                # cols 256:260 = s_src (a_j), cols 260:264 = s_dst (a_i)
                nc.vector.tensor_mul(tmp[:], wtile[:], arep_s[:, HC : 2 * HC])
                for h in range(H):
                    nc.vector.tensor_reduce(
                        wt_f[:, HC + h : HC + h + 1],
                        tmp[:, h * COUT : (h + 1) * COUT],
                        mybir.AxisListType.X,
                        mybir.AluOpType.add,
                    )
                nc.vector.tensor_mul(tmp[:], wtile[:], arep_s[:, 0:HC])
                for h in range(H):
                    nc.vector.tensor_reduce(
                        wt_f[:, HC + 4 + h : HC + 5 + h],
                        tmp[:, h * COUT : (h + 1) * COUT],
                        mybir.AxisListType.X,
                        mybir.AluOpType.add,
                    )
                nc.vector.tensor_copy(wt_ext[:], wt_f[:])

            # ---- Phase A: projection table ---------------------------------
            fat_v = xh_t.rearrange("(t p) f -> p t f", p=128)
            with (
                tc.tile_pool(name="pha", bufs=2) as pa,
                tc.tile_pool(name="psA", bufs=4, space="PSUM") as psA,
            ):
                for g in range(0, NTILES_A, GROUP_A):
                    gt = min(GROUP_A, NTILES_A - g)
                    stA = pa.tile([128, gt, FATU], BF16, tag="stA")
                    xt = None
                    for i in range(gt):
                        if i % 4 == 0:
                            xt = pa.tile([128, 4 * 128], F32R, tag="xt")
                            g0 = (g + i) * 128
                            xw = min(4 * 128, NPAD - g0)
                            nc.sync.dma_start(
                                xt[:, 0:xw], xT_in[:, g0 : g0 + xw]
                            )
                        ps = psA.tile([128, HC + 8], F32, tag="psA")
                        nc.tensor.matmul(
                            ps[:],
                            xt[:, (i % 4) * 128 : (i % 4 + 1) * 128],
                            wt_ext[:],
                            start=True,
                            stop=True,
                        )
                        # psum: [xh(0:256) | s_src(256:260) | s_dst(260:264)]
                        if i % 2 == 0:
                            nc.vector.tensor_copy(
                                stA[:, i, 0:HC], ps[:, 0:HC]
                            )
                        else:
                            nc.scalar.copy(stA[:, i, 0:HC], ps[:, 0:HC])
                        # s_src f32 bits into u16 cols 256:264 (raw f32 copy
                        # through a bitcast view: no conversion)
                        nc.vector.tensor_copy(
                            stA[:, i, HC : HC + 8].bitcast(F32),
                            ps[:, HC : HC + 4],
                        )
                        # finite filler for gathered-but-unused tail cols
                        nc.scalar.copy(
                            stA[:, i, HC + 8 : FATU].bitcast(F32),
                            ps[:, HC - 60 : HC],
                        )
                        # per-node s_dst stays resident in SBUF
                        if g + i < T:
                            nc.vector.tensor_copy(
                                sdst_sb[:, g + i, :], ps[:, HC + 4 : HC + 8]
                            )
                    nc.sync.dma_start(fat_v[:, g : g + gt, :], stA[:])

            # ---- Phase C: gather + scores + one-hot scatter matmul ---------
            y_v = y_out.rearrange("(t p) f -> p t f", p=128)
            if stage == "A":
                with tc.tile_pool(name="ost0", bufs=2) as ost0:
                    for tiles, fc_lo0, nlo, fc_hi0, nhi in st.groups:
                        og = ost0.tile([128, len(tiles), HC], F32, tag="og")
                        nc.vector.memset(og[:], 0.0)
                        nc.sync.dma_start(
                            y_v[:, tiles[0] : tiles[0] + len(tiles), :], og[:]
                        )
                nc.compile()
                return nc
            with (
                tc.tile_pool(name="phc", bufs=2) as pc,
                tc.tile_pool(name="mk", bufs=4) as mk,
                tc.tile_pool(name="pssg", bufs=2, space="PSUM") as psg,
                tc.tile_pool(name="pso", bufs=2, space="PSUM") as pso,
                tc.tile_pool(name="ost", bufs=2) as ost,
            ):
                for tiles, fc_lo0, nlo, fc_hi0, nhi in st.groups:
                    nall = nlo + nhi
                    # flat chunk -> tile map (lo block then hi block)
                    tflat = [t for t in tiles for _ in range(st.CL[t])] + [
                        t for t in tiles for _ in range(st.CH[t])
                    ]
                    # one-hot streams for this group's flat chunk range
                    Ps = pc.tile([128, nall * 128], BF16, tag="Ps")
                    nc.sync.dma_start(
                        Ps[:],
                        pm_in[:, fc_lo0 * 128 : (fc_lo0 + nall) * 128],
                    )
                    PTs = pc.tile([128, nall * 128], F32R, tag="PTs")
                    nc.sync.dma_start(
                        PTs[:],
                        ptm_in[:, fc_lo0 * 128 : (fc_lo0 + nall) * 128],
                    )
                    parts = []
                    if nlo:
                        glo = pc.tile([128, nlo, FATU], BF16, tag="glo")
                        nc.gpsimd.dma_gather(
                            glo[:],
                            xh_t[0:LO, :],
                            gidx_s[:, fc_lo0 * 8 : (fc_lo0 + nlo) * 8],
                            nlo * 128,
                            nlo * 128,
                            FATU,
                            single_packet=False,
                        )
                        parts.append((glo, 0, nlo))
                    if nhi:
                        ghi = pc.tile([128, nhi, FATU], BF16, tag="ghi")
                        nc.gpsimd.dma_gather(
                            ghi[:],
                            xh_t[LO:, :],
                            gidx_s[:, fc_hi0 * 8 : (fc_hi0 + nhi) * 8],
                            nhi * 128,
                            nhi * 128,
                            FATU,
                            single_packet=False,
                        )
                        parts.append((ghi, nlo, nhi))

                    # per-edge s_dst via one-hot transpose matmuls
                    pssg = psg.tile([128, nall, 4], F32, tag="pssg")
                    for j in range(nall):
                        nc.tensor.matmul(
                            pssg[:, j, :],
                            PTs[:, j * 128 : (j + 1) * 128],
                            sdst_sb[:, tflat[j], :],
                            start=True,
                            stop=True,
                            skip_group_check=True,
                        )

                    eg = pc.tile([128, nall, 4], F32, tag="eg")
                    for gt_, ofs, nch in parts:
                        # e = s_dst[dst] + s_src[src]
                        nc.vector.tensor_add(
                            eg[:, ofs : ofs + nch, :],
                            pssg[:, ofs : ofs + nch, :],
                            gt_[:, :, HC : HC + 8].bitcast(F32),
                        )
                    # leaky_relu: (e*0.2) max e
                    nc.vector.scalar_tensor_tensor(
                        eg[:],
                        eg[:],
                        NEG_SLOPE,
                        eg[:],
                        mybir.AluOpType.mult,
                        mybir.AluOpType.max,
                    )
                    for gt_, ofs, nch in parts:
                        # w = exp(e), written into G cols 256:260 (over the
                        # no-longer-needed s_src bits)
                        nc.scalar.activation(
                            gt_[:, :, HC : HC + 4],
                            eg[:, ofs : ofs + nch, :],
                            mybir.ActivationFunctionType.Exp,
                        )
                        # G[:, 0:256] *= w (bcast over 64)
                        nc.vector.tensor_mul(
                            gt_[:, :, 0:HC].rearrange(
                                "p c (h o) -> p c h o", o=COUT
                            ),
                            gt_[:, :, 0:HC].rearrange(
                                "p c (h o) -> p c h o", o=COUT
                            ),
                            gt_[:, :, HC : HC + 4]
                            .unsqueeze(-1)
                            .broadcast_to([128, nch, 4, COUT]),
                        )

                    og = ost.tile([128, len(tiles), HC], F32, tag="og")
                    lo_j = 0
                    hi_j = 0
                    for ti, t in enumerate(tiles):
                        ps = pso.tile([128, HC + 4], F32, tag="ps")
                        K = st.CL[t] + st.CH[t]
                        k = 0
                        for passno in range(2):
                            nch = st.CL[t] if passno == 0 else st.CH[t]
                            for _ in range(nch):
                                if passno == 0:
                                    flat = lo_j
                                    gsl = glo[:, lo_j, 0 : HC + 4]
                                    lo_j += 1
                                else:
                                    flat = nlo + hi_j
                                    gsl = ghi[:, hi_j, 0 : HC + 4]
                                    hi_j += 1
                                nc.tensor.matmul(
                                    ps[:],
                                    Ps[:, flat * 128 : (flat + 1) * 128],
                                    gsl,
                                    start=(k == 0),
                                    stop=(k == K - 1),
                                    skip_group_check=True,
                                )
                                k += 1
                        den = mk.tile([128, 4], F32, tag="den")
                        nc.vector.tensor_scalar(
                            den[:],
                            ps[:, HC : HC + 4],
                            1e-30,
                            None,
                            mybir.AluOpType.add,
                        )
                        rec = mk.tile([128, 4], F32, tag="rec")
                        nc.vector.reciprocal(rec[:], den[:])
                        nc.vector.tensor_mul(
                            og[:, ti, :].rearrange("p (h o) -> p h o", o=COUT),
                            ps[:, 0:HC].rearrange("p (h o) -> p h o", o=COUT),
                            rec.unsqueeze(-1).broadcast_to([128, 4, COUT]),
                        )
                    nc.sync.dma_start(
                        y_v[:, tiles[0] : tiles[0] + len(tiles), :], og[:]
                    )

    nc.compile()
    return nc


def _make_in_maps(st, bounds, x, W, a, gidx, Pm, PTm):
    xt_g = np.zeros((128, NPAD), np.float32)
    xt_g[:, :N_NODES] = np.ascontiguousarray(x.T)
    Wt = np.ascontiguousarray(W.transpose(2, 0, 1).reshape(CIN, HC)).astype(
        np.float32
    )
    arep = np.tile(
        np.concatenate([a[:, :COUT].reshape(-1), a[:, COUT:].reshape(-1)])[
            None, :
        ],
        (128, 1),
    ).astype(np.float32)

    in_maps = []
    for c in range(NCORES):
        n0, n1 = bounds[c], bounds[c + 1]
        nl = n1 - n0
        xTc = np.empty((128, NPAD), np.float32)
        xTc[:, :nl] = xt_g[:, n0:n1]
        xTc[:, nl : nl + n0] = xt_g[:, 0:n0]
        xTc[:, nl + n0 : nl + n0 + (NPAD - n1)] = xt_g[:, n1:NPAD]
        in_maps.append(
            {
                "xT": xTc,
                "Wt": Wt,
                "arep": arep,
                "gidx": np.ascontiguousarray(gidx[c]),
                "Pm": np.ascontiguousarray(Pm[c]),
                "PTm": np.ascontiguousarray(PTm[c]),
            }
        )
    return in_maps


_CACHE = {}


def _get_compiled(edge_key, edge_index):
    if edge_key not in _CACHE:
        st, bounds, nloc, gidx, Pm, PTm = _preprocess(edge_index)
        nc = _build_program(st)
        _CACHE[edge_key] = (st, bounds, nloc, gidx, Pm, PTm, nc)
    return _CACHE[edge_key]


def kernel(x, edge_index, W, a, num_nodes, _trace=False):
    x = np.asarray(x)
    edge_index = np.asarray(edge_index)
    W = np.asarray(W)
    a = np.asarray(a)

    edge_key = hash(edge_index.tobytes())
    st, bounds, nloc, gidx, Pm, PTm, nc = _get_compiled(edge_key, edge_index)
    in_maps = _make_in_maps(st, bounds, x, W, a, gidx, Pm, PTm)

    kw = {}
    if _trace:
        kw = dict(trace=True)
    res = run_bass_kernel_spmd(nc, in_maps, core_ids=list(range(NCORES)), **kw)

    out = np.empty((N_NODES, HC), np.float32)
    for c in range(NCORES):
        y = res.results[c]["y"]
        out[bounds[c] : bounds[c + 1]] = y[: nloc[c]]
    if _trace:
        return out, res
    return out


# revision 8
# speedup vs baseline: 1.0561x; 1.0473x over previous
"""Multi-head GAT layer on 8 Trainium2 NeuronCores.

Strategy (one SPMD program, 8 cores; per-core differences are data only):
  - Destination nodes are split into 8 contiguous, 128-aligned ranges
    balanced by edge count.  Each core computes out[] rows for its range.
  - Each core's inputs are supplied in a per-core *node permutation* that
    places its own (local) nodes first; all edge indices are pre-translated
    on the host (dma_gather indices are int16, hence a lo/hi table split).
  - Every core computes the full projection table with the tensor engine
    (replicated; avoids any collective):
       fat row [node, 384 bf16] = [xh bf16 (512B) | s_src f32 (16B) | pad]
    The per-node s_dst scores stay resident in SBUF (never round-trip DRAM).
  - Edges (with self loops) are grouped by destination tile (128 dsts) and
    padded to 128-edge chunks (one edge per SBUF partition).  Per chunk:
       G    = dma_gather(fat_table, src)          # 768B rows, the ONLY
                                                  # gpsimd-descriptor work
       sd_e = PT_chunk^T @ sdst_tile              # per-edge s_dst via a
                                                  # host-streamed one-hot
       w    = exp(leaky_relu(sd_e + G.s_src)) -> written into G[256:260]
       G[:, 0:256] *= w (bcast over 64)
       psum[128d, 260] += P_chunk^T @ G[:, 0:260] # host-streamed one-hot P
    Columns 0:256 accumulate the numerator, 256:260 the denominator
    (w columns summed by the same one-hot matmul).  out = num/den.
  - P (bf16) and PT (f32) matrices are precomputed on the host from the
    edge list and streamed as plain contiguous DMA: no on-device index
    math, no per-edge descriptor generation beyond the single fat gather.
  - exp() needs no max subtraction: scores are O(1) (inputs are standard
    normal), so overflow is impossible; softmax is shift-invariant.
"""

import sys

sys.path.insert(0, "/opt/trn_rl_repo")

import numpy as np
import ml_dtypes

import concourse.bass as bass
import concourse.mybir as mybir
from concourse import bacc, tile
from concourse.bass_utils import run_bass_kernel_spmd

# Problem constants (hardcoded per contest rules).
N_NODES = 50000
CIN = 128
COUT = 64
H = 4
HC = H * COUT  # 256
FATU = 384  # fat row in bf16/u16 units: xh(256) | s_src f32 as 8 u16 | pad
NEG_SLOPE = 0.2

NCORES = 8
TILE = 128
LO = 32768
NPAD = ((N_NODES + TILE - 1) // TILE) * TILE  # 50048
NTILES_A = NPAD // TILE  # 391
GROUP = 2  # dst tiles per phase-C group
GROUP_A = 16  # projection tiles per staging group

F32 = mybir.dt.float32
F32R = mybir.dt.float32r
BF16 = mybir.dt.bfloat16
I16 = mybir.dt.int16


def _cdiv(a, b):
    return (a + b - 1) // b


class Structure:
    def __init__(self, T, CL, CH, groups, TC):
        self.T = T
        self.CL = CL
        self.CH = CH
        self.groups = groups  # (tiles, fc_lo0, nlo, fc_hi0, nhi)
        self.TC = TC


def _preprocess(edge_index):
    """Host-side integer-only preprocessing."""
    src = edge_index[0].astype(np.int64)
    dst = edge_index[1].astype(np.int64)
    loops = np.arange(N_NODES, dtype=np.int64)
    row = np.concatenate([src, loops])
    col = np.concatenate([dst, loops])
    order = np.argsort(col, kind="stable")
    row = row[order]
    col = col[order]
    counts = np.bincount(col, minlength=N_NODES)
    ccum = np.concatenate([[0], np.cumsum(counts)])
    etot = row.size

    bounds = [0]
    for c in range(1, NCORES):
        n = int(np.searchsorted(ccum, etot * c // NCORES))
        n = (n // TILE) * TILE
        bounds.append(min(max(n, bounds[-1] + TILE), N_NODES - TILE))
    bounds.append(N_NODES)
    nloc = [bounds[c + 1] - bounds[c] for c in range(NCORES)]
    T = max(_cdiv(nl, TILE) for nl in nloc)

    # Per core: permuted-src + local-dst-offset edge lists per tile/pass.
    per_core = []
    for c in range(NCORES):
        n0, n1 = bounds[c], bounds[c + 1]
        e0, e1 = int(ccum[n0]), int(ccum[n1])
        r = row[e0:e1]
        d = col[e0:e1] - n0
        nl = n1 - n0
        # permuted source id: local nodes first, then [0,n0), then [n1,N)
        pr = np.where(
            (r >= n0) & (r < n1),
            r - n0,
            np.where(r < n0, nl + r, nl + n0 + (r - n1)),
        )
        # partition-major table row: node (tile t, part p) lives at row
        # p*NTILES_A + t, so phase A writes one contiguous run per partition
        pr = (pr % TILE) * NTILES_A + pr // TILE
        tt = d // TILE
        tiles = []
        for t in range(T):
            m = tt == t
            rs = pr[m]
            ds = d[m] - t * TILE
            lom = rs < LO
            tiles.append((rs[lom], ds[lom], rs[~lom] - LO, ds[~lom]))
        per_core.append(tiles)

    CL = [
        max(_cdiv(len(per_core[c][t][0]), TILE) for c in range(NCORES))
        for t in range(T)
    ]
    CH = [
        max(_cdiv(len(per_core[c][t][2]), TILE) for c in range(NCORES))
        for t in range(T)
    ]
    for t in range(T):
        if CL[t] + CH[t] == 0:
            CL[t] = 1

    groups = []
    fc = 0
    for g0 in range(0, T, GROUP):
        tiles = list(range(g0, min(g0 + GROUP, T)))
        nlo = sum(CL[t] for t in tiles)
        nhi = sum(CH[t] for t in tiles)
        groups.append((tiles, fc, nlo, fc + nlo, nhi))
        fc += nlo + nhi
    TC = fc

    st = Structure(T, CL, CH, groups, TC)

    gidx = np.zeros((NCORES, 128, TC * 8), np.int16)  # by permuted src
    # One-hot scatter (P) / broadcast (PT) matrices, streamed to the device.
    #   Pm[c][e, fc*128 + d] = 1  where chunk fc's edge e targets dst-off d
    #   PTm[c][d, fc*128 + e] = 1 (transpose layout, partition = dst-off)
    Pm = np.zeros((NCORES, 128, TC * 128), ml_dtypes.bfloat16)
    PTm = np.zeros((NCORES, 128, TC * 128), np.float32)

    wrow = np.arange(128) % 16
    wcol = np.arange(128) // 16

    for c in range(NCORES):
        p_e, p_col = [], []  # Pm fancy-index accumulators
        pt_d, pt_col = [], []
        for tiles, fc_lo0, nlo, fc_hi0, nhi in st.groups:
            fl, fh = fc_lo0, fc_hi0
            for t in tiles:
                lo_s, lo_d, hi_s, hi_d = per_core[c][t]
                for passno in range(2):
                    s_arr, d_arr = (lo_s, lo_d) if passno == 0 else (hi_s, hi_d)
                    nch = CL[t] if passno == 0 else CH[t]
                    base = fl if passno == 0 else fh
                    for k in range(nch):
                        sl = slice(k * TILE, (k + 1) * TILE)
                        sv = s_arr[sl]
                        dv = d_arr[sl]
                        m = len(sv)
                        fcx = base + k
                        if m:
                            for rep in range(8):
                                rr = rep * 16 + wrow[:m]
                                cc = fcx * 8 + wcol[:m]
                                gidx[c, rr, cc] = sv.astype(np.int16)
                            ep = np.arange(m)
                            p_e.append(ep)
                            p_col.append(fcx * 128 + dv)
                            pt_d.append(dv)
                            pt_col.append(fcx * 128 + ep)
                    if passno == 0:
                        fl += nch
                    else:
                        fh += nch
        if p_e:
            Pm[c][np.concatenate(p_e), np.concatenate(p_col)] = 1
            PTm[c][np.concatenate(pt_d), np.concatenate(pt_col)] = 1

    return st, bounds, nloc, gidx, Pm, PTm


def _build_program(st):
    import os

    stage = os.environ.get("GAT_STAGE", "full")
    nc = bacc.Bacc(None, target_bir_lowering=False)
    TC = st.TC
    T = st.T

    xT_in = nc.dram_tensor("xT", [128, NPAD], F32R, kind="ExternalInput")
    wt_in = nc.dram_tensor("Wt", [128, HC], F32, kind="ExternalInput")
    arep_in = nc.dram_tensor("arep", [128, 2 * HC], F32, kind="ExternalInput")
    gidx_in = nc.dram_tensor("gidx", [128, TC * 8], I16, kind="ExternalInput")
    pm_in = nc.dram_tensor("Pm", [128, TC * 128], BF16, kind="ExternalInput")
    ptm_in = nc.dram_tensor("PTm", [128, TC * 128], F32R, kind="ExternalInput")
    y_out = nc.dram_tensor("y", [T * 128, HC], F32, kind="ExternalOutput")

    with tile.TileContext(nc) as tc:
        with (
            tc.tile_pool(name="dram", bufs=1, space="DRAM") as dram,
            tc.tile_pool(name="persist", bufs=1) as pp,
        ):
            xh_t = dram.tile([NPAD, FATU], BF16)

            wt_ext = pp.tile([128, HC + 8], F32R)
            gidx_s = pp.tile([128, TC * 8], I16)
            nc.sync.dma_start(gidx_s[:], gidx_in[:])
            sdst_sb = pp.tile([128, T, 4], F32R)

            # ---- Wt_ext = [Wt | v_src(4) | v_dst(4)] -----------------------
            with tc.tile_pool(name="winit", bufs=1) as wini:
                wtile = wini.tile([128, HC], F32)
                nc.sync.dma_start(wtile[:], wt_in[:])
                arep_s = wini.tile([128, 2 * HC], F32)
                nc.sync.dma_start(arep_s[:], arep_in[:])
                tmp = wini.tile([128, HC], F32)
                wt_f = wini.tile([128, HC + 8], F32)
                nc.vector.tensor_copy(wt_f[:, 0:HC], wtile[:])
                # cols 256:260 = s_src (a_j), cols 260:264 = s_dst (a_i)
                nc.vector.tensor_mul(tmp[:], wtile[:], arep_s[:, HC : 2 * HC])
                for h in range(H):
                    nc.vector.tensor_reduce(
                        wt_f[:, HC + h : HC + h + 1],
                        tmp[:, h * COUT : (h + 1) * COUT],
                        mybir.AxisListType.X,
                        mybir.AluOpType.add,
                    )
                nc.vector.tensor_mul(tmp[:], wtile[:], arep_s[:, 0:HC])
                for h in range(H):
                    nc.vector.tensor_reduce(
                        wt_f[:, HC + 4 + h : HC + 5 + h],
                        tmp[:, h * COUT : (h + 1) * COUT],
                        mybir.AxisListType.X,
                        mybir.AluOpType.add,
                    )
                nc.vector.tensor_copy(wt_ext[:], wt_f[:])

            # ---- Phase A: projection table ---------------------------------
            # partition-major view: row p*NTILES_A + t <-> (p, t); per-group
            # writes are one contiguous per-partition run
            fat_v = xh_t.rearrange("(p t) f -> p t f", p=128)
            with (
                tc.tile_pool(name="pha", bufs=3) as pa,
                tc.tile_pool(name="psA", bufs=6, space="PSUM") as psA,
            ):
                for g in range(0, NTILES_A, GROUP_A):
                    gt = min(GROUP_A, NTILES_A - g)
                    stA = pa.tile([128, gt, FATU], BF16, tag="stA")
                    xt = None
                    for i in range(gt):
                        if i % 4 == 0:
                            xt = pa.tile([128, 4 * 128], F32R, tag="xt")
                            g0 = (g + i) * 128
                            xw = min(4 * 128, NPAD - g0)
                            nc.sync.dma_start(
                                xt[:, 0:xw], xT_in[:, g0 : g0 + xw]
                            )
                        ps = psA.tile([128, HC + 8], F32, tag="psA")
                        nc.tensor.matmul(
                            ps[:],
                            xt[:, (i % 4) * 128 : (i % 4 + 1) * 128],
                            wt_ext[:],
                            start=True,
                            stop=True,
                        )
                        # psum: [xh(0:256) | s_src(256:260) | s_dst(260:264)]
                        if i % 2 == 0:
                            nc.vector.tensor_copy(
                                stA[:, i, 0:HC], ps[:, 0:HC]
                            )
                        else:
                            nc.scalar.copy(stA[:, i, 0:HC], ps[:, 0:HC])
                        # tail cols 256:384 = raw f32 bits of ps[200:264]:
                        # finite junk filler, with s_src landing at u16 cols
                        # 368:376 (one copy, no conversion)
                        if i % 2 == 0:
                            nc.scalar.copy(
                                stA[:, i, HC:FATU].bitcast(F32),
                                ps[:, HC - 56 : HC + 8],
                            )
                        else:
                            nc.vector.tensor_copy(
                                stA[:, i, HC:FATU].bitcast(F32),
                                ps[:, HC - 56 : HC + 8],
                            )
                        # per-node s_dst stays resident in SBUF
                        if g + i < T:
                            nc.vector.tensor_copy(
                                sdst_sb[:, g + i, :], ps[:, HC + 4 : HC + 8]
                            )
                    nc.sync.dma_start(fat_v[:, g : g + gt, :], stA[:])

            # ---- Phase C: gather + scores + one-hot scatter matmul ---------
            y_v = y_out.rearrange("(t p) f -> p t f", p=128)
            if stage == "A":
                with tc.tile_pool(name="ost0", bufs=2) as ost0:
                    for tiles, fc_lo0, nlo, fc_hi0, nhi in st.groups:
                        og = ost0.tile([128, len(tiles), HC], F32, tag="og")
                        nc.vector.memset(og[:], 0.0)
                        nc.sync.dma_start(
                            y_v[:, tiles[0] : tiles[0] + len(tiles), :], og[:]
                        )
                nc.compile()
                return nc
            with (
                tc.tile_pool(name="phc", bufs=2) as pc,
                tc.tile_pool(name="mk", bufs=4) as mk,
                tc.tile_pool(name="pssg", bufs=2, space="PSUM") as psg,
                tc.tile_pool(name="pso", bufs=2, space="PSUM") as pso,
                tc.tile_pool(name="ost", bufs=2) as ost,
            ):
                for tiles, fc_lo0, nlo, fc_hi0, nhi in st.groups:
                    nall = nlo + nhi
                    # flat chunk -> tile map (lo block then hi block)
                    tflat = [t for t in tiles for _ in range(st.CL[t])] + [
                        t for t in tiles for _ in range(st.CH[t])
                    ]
                    # one-hot streams for this group's flat chunk range
                    Ps = pc.tile([128, nall * 128], BF16, tag="Ps")
                    nc.sync.dma_start(
                        Ps[:],
                        pm_in[:, fc_lo0 * 128 : (fc_lo0 + nall) * 128],
                    )
                    PTs = pc.tile([128, nall * 128], F32R, tag="PTs")
                    nc.sync.dma_start(
                        PTs[:],
                        ptm_in[:, fc_lo0 * 128 : (fc_lo0 + nall) * 128],
                    )
                    parts = []
                    if nlo:
                        glo = pc.tile([128, nlo, FATU], BF16, tag="glo")
                        nc.gpsimd.dma_gather(
                            glo[:],
                            xh_t[0:LO, :],
                            gidx_s[:, fc_lo0 * 8 : (fc_lo0 + nlo) * 8],
                            nlo * 128,
                            nlo * 128,
                            FATU,
                            single_packet=False,
                        )
                        parts.append((glo, 0, nlo))
                    if nhi:
                        ghi = pc.tile([128, nhi, FATU], BF16, tag="ghi")
                        nc.gpsimd.dma_gather(
                            ghi[:],
                            xh_t[LO:, :],
                            gidx_s[:, fc_hi0 * 8 : (fc_hi0 + nhi) * 8],
                            nhi * 128,
                            nhi * 128,
                            FATU,
                            single_packet=False,
                        )
                        parts.append((ghi, nlo, nhi))

                    # per-edge s_dst via one-hot transpose matmuls
                    pssg = psg.tile([128, nall, 4], F32, tag="pssg")
                    for j in range(nall):
                        nc.tensor.matmul(
                            pssg[:, j, :],
                            PTs[:, j * 128 : (j + 1) * 128],
                            sdst_sb[:, tflat[j], :],
                            start=True,
                            stop=True,
                            skip_group_check=True,
                        )

                    eg = pc.tile([128, nall, 4], F32, tag="eg")
                    for gt_, ofs, nch in parts:
                        # e = s_dst[dst] + s_src[src]
                        nc.vector.tensor_add(
                            eg[:, ofs : ofs + nch, :],
                            pssg[:, ofs : ofs + nch, :],
                            gt_[:, :, FATU - 16 : FATU - 8].bitcast(F32),
                        )
                    # leaky_relu: (e*0.2) max e
                    nc.vector.scalar_tensor_tensor(
                        eg[:],
                        eg[:],
                        NEG_SLOPE,
                        eg[:],
                        mybir.AluOpType.mult,
                        mybir.AluOpType.max,
                    )
                    for gt_, ofs, nch in parts:
                        # w = exp(e), written into G cols 256:260 (over the
                        # no-longer-needed s_src bits)
                        nc.scalar.activation(
                            gt_[:, :, HC : HC + 4],
                            eg[:, ofs : ofs + nch, :],
                            mybir.ActivationFunctionType.Exp,
                        )
                        # G[:, 0:256] *= w (bcast over 64)
                        nc.vector.tensor_mul(
                            gt_[:, :, 0:HC].rearrange(
                                "p c (h o) -> p c h o", o=COUT
                            ),
                            gt_[:, :, 0:HC].rearrange(
                                "p c (h o) -> p c h o", o=COUT
                            ),
                            gt_[:, :, HC : HC + 4]
                            .unsqueeze(-1)
                            .broadcast_to([128, nch, 4, COUT]),
                        )

                    og = ost.tile([128, len(tiles), HC], F32, tag="og")
                    lo_j = 0
                    hi_j = 0
                    for ti, t in enumerate(tiles):
                        ps = pso.tile([128, HC + 4], F32, tag="ps")
                        K = st.CL[t] + st.CH[t]
                        k = 0
                        for passno in range(2):
                            nch = st.CL[t] if passno == 0 else st.CH[t]
                            for _ in range(nch):
                                if passno == 0:
                                    flat = lo_j
                                    gsl = glo[:, lo_j, 0 : HC + 4]
                                    lo_j += 1
                                else:
                                    flat = nlo + hi_j
                                    gsl = ghi[:, hi_j, 0 : HC + 4]
                                    hi_j += 1
                                nc.tensor.matmul(
                                    ps[:],
                                    Ps[:, flat * 128 : (flat + 1) * 128],
                                    gsl,
                                    start=(k == 0),
                                    stop=(k == K - 1),
                                    skip_group_check=True,
                                )
                                k += 1
                        den = mk.tile([128, 4], F32, tag="den")
                        nc.vector.tensor_scalar(
                            den[:],
                            ps[:, HC : HC + 4],
                            1e-30,
                            None,
                            mybir.AluOpType.add,
                        )
                        rec = mk.tile([128, 4], F32, tag="rec")
                        nc.vector.reciprocal(rec[:], den[:])
                        nc.vector.tensor_mul(
                            og[:, ti, :].rearrange("p (h o) -> p h o", o=COUT),
                            ps[:, 0:HC].rearrange("p (h o) -> p h o", o=COUT),
                            rec.unsqueeze(-1).broadcast_to([128, 4, COUT]),
                        )
                    nc.sync.dma_start(
                        y_v[:, tiles[0] : tiles[0] + len(tiles), :], og[:]
                    )

    nc.compile()
    return nc


def _make_in_maps(st, bounds, x, W, a, gidx, Pm, PTm):
    xt_g = np.zeros((128, NPAD), np.float32)
    xt_g[:, :N_NODES] = np.ascontiguousarray(x.T)
    Wt = np.ascontiguousarray(W.transpose(2, 0, 1).reshape(CIN, HC)).astype(
        np.float32
    )
    arep = np.tile(
        np.concatenate([a[:, :COUT].reshape(-1), a[:, COUT:].reshape(-1)])[
            None, :
        ],
        (128, 1),
    ).astype(np.float32)

    in_maps = []
    for c in range(NCORES):
        n0, n1 = bounds[c], bounds[c + 1]
        nl = n1 - n0
        xTc = np.empty((128, NPAD), np.float32)
        xTc[:, :nl] = xt_g[:, n0:n1]
        xTc[:, nl : nl + n0] = xt_g[:, 0:n0]
        xTc[:, nl + n0 : nl + n0 + (NPAD - n1)] = xt_g[:, n1:NPAD]
        in_maps.append(
            {
                "xT": xTc,
                "Wt": Wt,
                "arep": arep,
                "gidx": np.ascontiguousarray(gidx[c]),
                "Pm": np.ascontiguousarray(Pm[c]),
                "PTm": np.ascontiguousarray(PTm[c]),
            }
        )
    return in_maps


_CACHE = {}


def _get_compiled(edge_key, edge_index):
    if edge_key not in _CACHE:
        st, bounds, nloc, gidx, Pm, PTm = _preprocess(edge_index)
        nc = _build_program(st)
        _CACHE[edge_key] = (st, bounds, nloc, gidx, Pm, PTm, nc)
    return _CACHE[edge_key]


def kernel(x, edge_index, W, a, num_nodes, _trace=False):
    x = np.asarray(x)
    edge_index = np.asarray(edge_index)
    W = np.asarray(W)
    a = np.asarray(a)

    edge_key = hash(edge_index.tobytes())
    st, bounds, nloc, gidx, Pm, PTm, nc = _get_compiled(edge_key, edge_index)
    in_maps = _make_in_maps(st, bounds, x, W, a, gidx, Pm, PTm)

    kw = {}
    if _trace:
        kw = dict(trace=True)
    res = run_bass_kernel_spmd(nc, in_maps, core_ids=list(range(NCORES)), **kw)

    out = np.empty((N_NODES, HC), np.float32)
    for c in range(NCORES):
        y = res.results[c]["y"]
        out[bounds[c] : bounds[c + 1]] = y[: nloc[c]]
    if _trace:
        return out, res
    return out
